# revision 39
# baseline (speedup 1.0000x reference)
"""Distributed GQA attention (B=2, S=2048, H=2048, 32 heads / 8 KV heads,
RoPE, causal) on 8 TRN2 NeuronCores.

Sharding: core c -> (batch b = c//4, head-group hg = c%4).
Each core computes q-heads [8hg, 8hg+8) and kv-heads [2hg, 2hg+2) of its
batch, runs attention locally (GQA groups stay on-core), then the four
cores of a batch AllGather their attention outputs (bf16) and each
computes a disjoint 512-column slice of the output projection.

Schedule (v2): attention runs kt-outer over 512-wide query chunks so
score/AV matmuls stream 512 columns (full PE rate) with exact causal
widths; softmax masking is a binary multiply on the Pool engine after
exp; the per-head normalization uses reciprocal_approx_fast and runs on
Pool.  Q-projection chunks and O-projection work are interleaved into
attention's scalar-bound bubbles, and the AllGather is split into four
512-column chunks so only the last O-proj chunk is a serial tail.
"""
import os
import sys

sys.path.insert(0, "/opt/trn_rl_repo")

import numpy as np
import ml_dtypes

import concourse.bass as bass
import concourse.mybir as mybir
import concourse.tile as tile
from concourse import bacc
from concourse import bass_utils

BF16 = mybir.dt.bfloat16
F32 = mybir.dt.float32
ADD = mybir.AluOpType.add
MULT = mybir.AluOpType.mult

B, S, H = 2, 2048, 2048
NH, NKV, HD = 32, 8, 64
SCALE = HD ** -0.5
RG = [[0, 1, 2, 3], [4, 5, 6, 7]]
N_CORES = 8
NT = S // 128          # 16 seq tiles
HT = H // 128          # 16 hidden tiles

TRACE = os.environ.get("BASS_KERNEL_TRACE", "0") == "1"
LAST_EXEC_NS = None
_COMPILED = None


def _install_profile_shim():
    import types
    try:
        from trn_agent_boot.trn_boot import _ntff_profile_via_ctypes
    except ImportError:
        return
    hook = _ntff_profile_via_ctypes("/opt/axon/libaxon_pjrt.so")
    mod = types.ModuleType("antenv.axon_hooks")
    mod.get_axon_ntff_profile_hook = lambda: hook
    mod.set_axon_ntff_profile_hook = lambda h: None
    sys.modules["antenv.axon_hooks"] = mod
    bass_utils.upload_artifacts = lambda tmpdir: tmpdir


def _build():
    nc = bacc.Bacc("TRN2", target_bir_lowering=False, debug=False,
                   num_devices=N_CORES)

    xt = nc.dram_tensor("xt", [H, S], BF16, kind="ExternalInput")
    wqt = nc.dram_tensor("wqt", [H, 512], BF16, kind="ExternalInput")
    wkt = nc.dram_tensor("wkt", [H, 128], BF16, kind="ExternalInput")
    wvt = nc.dram_tensor("wvt", [H, 128], BF16, kind="ExternalInput")
    wot = nc.dram_tensor("wot", [H, 512], BF16, kind="ExternalInput")
    bq = nc.dram_tensor("bq", [512, 1], F32, kind="ExternalInput")
    bk = nc.dram_tensor("bk", [128, 1], F32, kind="ExternalInput")
    bvrep = nc.dram_tensor("bvrep", [128, 128], F32, kind="ExternalInput")
    bo = nc.dram_tensor("bo", [512, 1], F32, kind="ExternalInput")
    qcos = nc.dram_tensor("qcos", [128, S], BF16, kind="ExternalInput")
    qsin = nc.dram_tensor("qsin", [128, S], BF16, kind="ExternalInput")
    kcos = nc.dram_tensor("kcos", [128, S], BF16, kind="ExternalInput")
    ksin = nc.dram_tensor("ksin", [128, S], BF16, kind="ExternalInput")
    mska = nc.dram_tensor("mska", [128, 128], BF16, kind="ExternalInput")
    msknb = nc.dram_tensor("msknb", [128, 128], BF16, kind="ExternalInput")
    out = nc.dram_tensor("out", [512, S], F32, kind="ExternalOutput")

    Exp = mybir.ActivationFunctionType.Exp

    from contextlib import ExitStack
    with tile.TileContext(nc) as tc:
        with ExitStack() as stk:
            ep = stk.enter_context
            big = ep(tc.tile_pool(name="big", bufs=16))     # xt / gathered o
            wpool = ep(tc.tile_pool(name="w", bufs=16))     # wqt / wot
            wkpool = ep(tc.tile_pool(name="wk", bufs=16))
            wvpool = ep(tc.tile_pool(name="wv", bufs=16))
            qpool = ep(tc.tile_pool(name="qt", bufs=4))
            kpool = ep(tc.tile_pool(name="kt", bufs=5))
            vpool = ep(tc.tile_pool(name="vv", bufs=16))
            opool = ep(tc.tile_pool(name="ot", bufs=4))
            tabpool = ep(tc.tile_pool(name="tab", bufs=4))
            mkpool = ep(tc.tile_pool(name="mk", bufs=1))
            ropepool = ep(tc.tile_pool(name="rope", bufs=6))
            expool = ep(tc.tile_pool(name="exp", bufs=4))
            nrmpool = ep(tc.tile_pool(name="nrm", bufs=2))
            ypool = ep(tc.tile_pool(name="yy", bufs=2))
            bpool = ep(tc.tile_pool(name="bias", bufs=12))
            pp = ep(tc.tile_pool(name="pp", bufs=2, space="PSUM"))
            scp = ep(tc.tile_pool(name="sc", bufs=2, space="PSUM"))
            avp = ep(tc.tile_pool(name="av", bufs=4, space="PSUM"))
            dram = ep(tc.tile_pool(name="dram", bufs=1, space="DRAM"))

            # ---------- input loads (weights for K/V first, then xt) ----------
            wk_sb, wv_sb = [], []
            for t in range(HT):
                k_t = wkpool.tile([128, 128], BF16, name=f"wk{t}", tag="wk")
                nc.sync.dma_start(out=k_t[:, :], in_=wkt[128 * t:128 * (t + 1), :])
                wk_sb.append(k_t)
                v_t = wvpool.tile([128, 128], BF16, name=f"wv{t}", tag="wv")
                nc.sync.dma_start(out=v_t[:, :], in_=wvt[128 * t:128 * (t + 1), :])
                wv_sb.append(v_t)
            xt_sb = []
            for t in range(HT):
                x_t = big.tile([128, S], BF16, name=f"xt{t}", tag="big")
                nc.sync.dma_start(out=x_t[:, :], in_=xt[128 * t:128 * (t + 1), :])
                xt_sb.append(x_t)
            wq_sb = []
            for t in range(HT):
                q_t = wpool.tile([128, 512], BF16, name=f"wq{t}", tag="w")
                nc.sync.dma_start(out=q_t[:, :], in_=wqt[128 * t:128 * (t + 1), :])
                wq_sb.append(q_t)
            kcos_sb = tabpool.tile([128, S], BF16, name="kcos", tag="tab")
            nc.sync.dma_start(out=kcos_sb[:, :], in_=kcos[:, :])
            ksin_sb = tabpool.tile([128, S], BF16, name="ksin", tag="tab")
            nc.sync.dma_start(out=ksin_sb[:, :], in_=ksin[:, :])
            qcos_sb = tabpool.tile([128, S], BF16, name="qcos", tag="tab")
            nc.sync.dma_start(out=qcos_sb[:, :], in_=qcos[:, :])
            qsin_sb = tabpool.tile([128, S], BF16, name="qsin", tag="tab")
            nc.sync.dma_start(out=qsin_sb[:, :], in_=qsin[:, :])
            mska_sb = mkpool.tile([128, 128], BF16, name="mska", tag="mka")
            nc.sync.dma_start(out=mska_sb[:, :], in_=mska[:, :])
            msknb_sb = mkpool.tile([128, 128], BF16, name="msknb", tag="mkb")
            nc.sync.dma_start(out=msknb_sb[:, :], in_=msknb[:, :])
            bq_sb, bo_sb = [], []
            for o in range(4):
                b_t = bpool.tile([128, 1], F32, name=f"bq{o}", tag="bias")
                nc.sync.dma_start(out=b_t[:, :], in_=bq[128 * o:128 * (o + 1), :])
                bq_sb.append(b_t)
            bk_sb = bpool.tile([128, 1], F32, name="bk", tag="bias")
            nc.sync.dma_start(out=bk_sb[:, :], in_=bk[:, :])
            bvrep_sb = bpool.tile([128, 128], F32, name="bvrep", tag="bvrep")
            nc.sync.dma_start(out=bvrep_sb[:, :], in_=bvrep[:, :])
            for o in range(4):
                b_t = bpool.tile([128, 1], F32, name=f"bo{o}", tag="bias")
                nc.sync.dma_start(out=b_t[:, :], in_=bo[128 * o:128 * (o + 1), :])
                bo_sb.append(b_t)

            def rope(psum, bias_ap, cos_sb, sin_sb, c, out_ap,
                     dve_shift=False):
                """out = (psum+bias)*cos + shift32((psum+bias)*sin_pre).

                dve_shift: do the rotate-half shift as 4 partition-shifted
                DVE adds instead of SBUF-SBUF DMAs — used for ropes emitted
                during attention, where the shift DMAs would queue behind
                AllGather traffic and stall the in-order PE queue."""
                cs = slice(512 * c, 512 * (c + 1))
                tcos = ropepool.tile([128, 512], F32, name="tcos", tag="rope")
                nc.vector.scalar_tensor_tensor(
                    tcos[:, :], psum[:, :], bias_ap, cos_sb[:, cs],
                    op0=ADD, op1=MULT)
                if dve_shift:
                    tsh = ropepool.tile([128, 512], F32, name="tsh",
                                        tag="rope")
                    for d, s in ((0, 32), (32, 0), (64, 96), (96, 64)):
                        nc.vector.scalar_tensor_tensor(
                            tsh[d:d + 32, :], psum[s:s + 32, :],
                            bias_ap[s:s + 32, :], sin_sb[s:s + 32, cs],
                            op0=ADD, op1=MULT)
                    nc.vector.tensor_tensor(out_ap, tcos[:, :], tsh[:, :],
                                            ADD)
                    return
                tsin = ropepool.tile([128, 512], F32, name="tsin", tag="rope")
                nc.vector.scalar_tensor_tensor(
                    tsin[:, :], psum[:, :], bias_ap, sin_sb[:, cs],
                    op0=ADD, op1=MULT)
                tsh = ropepool.tile([128, 512], F32, name="tsh", tag="rope")
                for d, s in ((0, 32), (32, 0), (64, 96), (96, 64)):
                    nc.sync.dma_start(out=tsh[d:d + 32, :], in_=tsin[s:s + 32, :])
                nc.vector.tensor_tensor(out_ap, tcos[:, :], tsh[:, :], ADD)

            # ---------- K projection (t-outer: streams behind the xt DMA) ----
            kp = []
            for c in range(4):
                pool = avp if c < 2 else pp
                kp.append(pool.tile([128, 512], F32, name=f"psk{c}",
                                    tag="av" if c < 2 else "pp"))
            for t in range(HT):
                for c in range(4):
                    nc.tensor.matmul(kp[c][:, :], wk_sb[t][:, :],
                                     xt_sb[t][:, 512 * c:512 * (c + 1)],
                                     start=(t == 0), stop=(t == HT - 1))
            kT_sb = kpool.tile([128, S], BF16, name="kT", tag="kt")
            # zero-padded K per (kv head, row band): [K_g; 0] for even heads,
            # [0; K_g] for odd heads.  Score matmuls then contract the full
            # 128 partitions of qT (the other head's rows hit the zero band),
            # streaming at full PE rate instead of the 64-deep half rate.
            # Memsets are emitted first: the DVE is idle during the input
            # DMA, and they must not queue behind the rope chain.
            kTe, kTo = [], []
            for g in range(2):
                e_t = kpool.tile([128, S], BF16, name=f"kTe{g}", tag="kt")
                nc.vector.memset(e_t[64:128, :], 0.0)
                kTe.append(e_t)
                o_t = kpool.tile([128, S], BF16, name=f"kTo{g}", tag="kt")
                nc.vector.memset(o_t[0:64, :], 0.0)
                kTo.append(o_t)
            for c in range(4):
                rope(kp[c], bk_sb[:, :], kcos_sb, ksin_sb, c,
                     kT_sb[:, 512 * c:512 * (c + 1)])
            for g in range(2):
                nc.sync.dma_start(out=kTe[g][0:64, :],
                                  in_=kT_sb[64 * g:64 * g + 64, :])
                nc.sync.dma_start(out=kTo[g][64:128, :],
                                  in_=kT_sb[64 * g:64 * g + 64, :])

            # ---------- Q projection (per 512-col chunk, generator) ----------
            qT_sb = []
            for o in range(4):
                q_t = qpool.tile([128, S], BF16, name=f"qT{o}", tag="qt")
                qT_sb.append(q_t)

            def qproj_gen(c):
                for o in range(4):
                    # chunk 0 runs at startup while pp slots still wait on
                    # the K-proj ropes for c2/c3 — alternate pools there.
                    pool, tg = ((avp, "av") if c == 0 and o < 2 else
                                (pp, "pp"))
                    psq = pool.tile([128, 512], F32, name=f"psq{c}_{o}",
                                    tag=tg)
                    for t in range(HT):
                        nc.tensor.matmul(psq[:, :],
                                         wq_sb[t][:, 128 * o:128 * (o + 1)],
                                         xt_sb[t][:, 512 * c:512 * (c + 1)],
                                         start=(t == 0), stop=(t == HT - 1))
                        yield
                    rope(psq, bq_sb[o][:, :], qcos_sb, qsin_sb, c,
                         qT_sb[o][:, 512 * c:512 * (c + 1)],
                         dve_shift=(c != 0))
                    yield

            def drain(gen):
                for _ in gen:
                    pass

            # ---------- V projection (layout [rows, oc], 65-strided + ones) --
            v_sb = []

            def vproj_gen():
                for rt in range(NT):
                    v_t = vpool.tile([128, 130], BF16, name=f"v{rt}", tag="v")
                    nc.vector.memset(
                        v_t[:, :].rearrange("p (m c) -> p m c", c=65)[:, :, 64:65], 1.0)
                    ps = scp.tile([128, 128], F32, name="psv", tag="sc")
                    for t in range(HT):
                        nc.tensor.matmul(ps[:, :],
                                         xt_sb[t][:, 128 * rt:128 * (rt + 1)],
                                         wv_sb[t][:, :],
                                         start=(t == 0), stop=(t == HT - 1))
                    # v + bv: with normalized softmax weights, adding bv to V
                    # is exactly the attention-output V-bias.
                    nc.vector.tensor_tensor(
                        v_t[:, :].rearrange("p (m c) -> p m c", c=65)[:, :, 0:64],
                        ps[:, :].rearrange("p (m c) -> p m c", c=64),
                        bvrep_sb[:, :].rearrange("p (m c) -> p m c", c=64), ADD)
                    v_sb.append(v_t)
                    yield

            # round-robin Q chunk 0 with V so the DVE rope-Q and V-bias work
            # interleave and early v tiles are ready when attention starts
            gens0 = [qproj_gen(0), vproj_gen()]
            while gens0:
                for g in list(gens0):
                    try:
                        next(g)
                    except StopIteration:
                        gens0.remove(g)

            # ---------- attention + output projection ----------
            oT_sb = []
            for j in range(4):
                o_t = opool.tile([128, S], BF16, name=f"oT{j}", tag="ot")
                oT_sb.append(o_t)

            bounce = [dram.tile([512, 512], BF16, name=f"bounce{g}")
                      for g in range(4)]
            gath = [dram.tile([4, 512, 512], BF16, name=f"gath{g}")
                    for g in range(4)]
            wo_sb = [None] * HT

            def wo_gen():
                for t in range(HT):
                    w_t = wpool.tile([128, 512], BF16, name=f"wo{t}", tag="w")
                    nc.sync.dma_start(
                        out=w_t[:, :], in_=wot[128 * t:128 * (t + 1), :])
                    wo_sb[t] = w_t
                    yield

            def emit_gather(g):
                for jj in range(4):
                    nc.sync.dma_start(
                        out=bounce[g][128 * jj:128 * (jj + 1), :],
                        in_=oT_sb[jj][:, 512 * g:512 * (g + 1)])
                nc.gpsimd.collective_compute(
                    "AllGather", mybir.AluOpType.bypass, replica_groups=RG,
                    ins=[bounce[g][:, :].opt()],
                    outs=[gath[g][:, :, :].opt()])

            def oproj_gen(g):
                gview = gath[g][:, :, :].rearrange("g i q -> (g i) q")
                gsb = []
                for t in range(HT):
                    g_t = big.tile([128, 512], BF16, name=f"g{g}_{t}", tag="big")
                    nc.sync.dma_start(out=g_t[:, :],
                                      in_=gview[128 * t:128 * (t + 1), :])
                    gsb.append(g_t)
                yield
                for o in range(4):
                    ps = pp.tile([128, 512], F32, name=f"psy{g}_{o}", tag="pp")
                    for t in range(HT):
                        nc.tensor.matmul(
                            ps[:, :], wo_sb[t][:, 128 * o:128 * (o + 1)],
                            gsb[t][:, :],
                            start=(t == 0), stop=(t == HT - 1))
                        yield
                    y_t = ypool.tile([128, 512], F32, name="y", tag="y")
                    nc.vector.tensor_scalar_add(y_t[:, :], ps[:, :],
                                                bo_sb[o][:, :])
                    nc.sync.dma_start(
                        out=out[128 * o:128 * (o + 1),
                                512 * g:512 * (g + 1)],
                        in_=y_t[:, :])
                    yield

            def epilogue(j, c, av_e, av_o):
                cs = slice(512 * c, 512 * (c + 1))
                # partition shifts must be 32-aligned: land den_e at row 32
                # and den_o at row 64 of one tile (other rows are av values,
                # initialized but unused), then one reciprocal covers both.
                den = nrmpool.tile([65, 512], F32, name="den", tag="den")
                nc.vector.tensor_copy(den[0:64, :], av_e[0:64, :])    # filler
                nc.vector.tensor_copy(den[0:1, :], av_e[64:65, :])    # e @ 0
                nc.vector.tensor_copy(den[64:65, :], av_o[64:65, :])  # o @ 64
                rec = nrmpool.tile([65, 512], F32, name="rec", tag="rec")
                nc.vector.reciprocal(rec[:, :], den[:, :])
                # partition_broadcast reads partition 0 of its source on hw:
                # stage the o-head reciprocal down to partition 0.
                rec2 = nrmpool.tile([1, 512], F32, name="rec2", tag="rec2")
                nc.vector.tensor_copy(rec2[0:1, :], rec[64:65, :])
                for par, avt in ((0, av_e), (1, av_o)):
                    bcast = nrmpool.tile([64, 512], F32, name="bcast",
                                         tag="bcast")
                    nc.gpsimd.partition_broadcast(bcast[:, :],
                                                  rec[0:1, :] if par == 0
                                                  else rec2[0:1, :],
                                                  channels=64)
                    nc.vector.tensor_tensor(
                        oT_sb[j][64 * par:64 * par + 64, cs],
                        avt[0:64, :], bcast[:, :], MULT)

            def attn_chunk(c, gens):
                """Attention for q columns [512c, 512c+512), kt-outer.
                gens: generators of extra work interleaved into the PE
                bubbles (attention here is scalar-engine bound)."""
                n_iters = 4 * (4 * c + 4)
                n_done = 0

                def interleave(frac_done):
                    nonlocal n_done
                    # keep emitted fraction of gens ~ fraction of iters done
                    while gens and n_done < frac_done * _gen_total[0]:
                        try:
                            next(gens[0])
                            n_done += 1
                        except StopIteration:
                            gens.pop(0)

                _gen_total = [0]
                # count yields cheaply: assume caller passes (gen, size) pairs
                sized = gens
                gens = [g for g, n in sized]
                _gen_total[0] = sum(n for g, n in sized)

                it = 0
                last = 4 * c + 3
                for j in range(4):
                    kv = j // 2
                    ke = kTe[kv]
                    ko = kTo[kv]
                    av_e = avp.tile([65, 512], F32, name=f"av{c}{j}e", tag="av")
                    av_o = avp.tile([65, 512], F32, name=f"av{c}{j}o", tag="av")
                    for kt in range(last + 1):
                        off = 128 * (kt - 4 * c) if kt >= 4 * c else 0
                        w = 512 - off
                        qsl = slice(512 * c + off, 512 * (c + 1))
                        ks = slice(128 * kt, 128 * (kt + 1))
                        diag = kt >= 4 * c
                        se = scp.tile([128, 512], F32, name="se", tag="sc")
                        so = scp.tile([128, 512], F32, name="so", tag="sc")
                        nc.tensor.matmul(se[:, 0:w], ke[:, ks],
                                         qT_sb[j][:, qsl],
                                         start=True, stop=not diag)
                        if diag:
                            # additive causal mask of the diagonal 128x128
                            # block as a matmul: (A^T B)[k,q] = NEG iff k > q
                            nc.tensor.matmul(se[:, 0:128], mska_sb[:, :],
                                             msknb_sb[:, :],
                                             start=False, stop=True)
                        nc.tensor.matmul(so[:, 0:w], ko[:, ks],
                                         qT_sb[j][:, qsl],
                                         start=True, stop=not diag)
                        if diag:
                            nc.tensor.matmul(so[:, 0:128], mska_sb[:, :],
                                             msknb_sb[:, :],
                                             start=False, stop=True)
                        ebuf = expool.tile([128, 512], BF16, name="ebuf",
                                           tag="exp")
                        obuf = expool.tile([128, 512], BF16, name="obuf",
                                           tag="exp")
                        nc.scalar.activation(ebuf[:, 0:w], se[:, 0:w], Exp)
                        nc.scalar.activation(obuf[:, 0:w], so[:, 0:w], Exp)
                        vs = v_sb[kt][:, 65 * kv:65 * kv + 65]
                        nc.tensor.matmul(av_e[:, off:512], vs, ebuf[:, 0:w],
                                         start=(kt == 0), stop=(kt == last))
                        nc.tensor.matmul(av_o[:, off:512], vs, obuf[:, 0:w],
                                         start=(kt == 0), stop=(kt == last))
                        it += 1
                        interleave(it / n_iters)
                    epilogue(j, c, av_e, av_o)
                # drain leftovers
                for g in gens:
                    drain(g)

            QP_N = 4 * (HT + 1)          # yields in qproj_gen
            OP_N = 1 + 4 * (HT + 1)      # yields in oproj_gen
            attn_chunk(0, [(qproj_gen(1), QP_N)])
            emit_gather(0)
            attn_chunk(1, [(qproj_gen(2), QP_N)])
            emit_gather(1)
            attn_chunk(2, [(qproj_gen(3), QP_N), (wo_gen(), HT)])
            emit_gather(2)
            attn_chunk(3, [(oproj_gen(0), OP_N)])
            drain(oproj_gen(1))
            drain(oproj_gen(2))
            emit_gather(3)
            drain(oproj_gen(3))

    nc.compile()
    return nc


def kernel(**inputs):
    global _COMPILED, LAST_EXEC_NS
    x = np.asarray(inputs["hidden_states"], dtype=np.float32)
    mask = np.asarray(inputs["attention_mask"], dtype=np.float32)
    pos = np.asarray(inputs["position_ids"])
    Wq = np.asarray(inputs["Wq"], dtype=np.float32)
    bq = np.asarray(inputs["bq"], dtype=np.float32)
    Wk = np.asarray(inputs["Wk"], dtype=np.float32)
    bk = np.asarray(inputs["bk"], dtype=np.float32)
    Wv = np.asarray(inputs["Wv"], dtype=np.float32)
    bv = np.asarray(inputs["bv"], dtype=np.float32)
    Wo = np.asarray(inputs["Wo"], dtype=np.float32)
    bo = np.asarray(inputs["bo"], dtype=np.float32)

    bf = ml_dtypes.bfloat16
    # rope tables (from the position_ids input)
    p = pos[0].astype(np.float32)
    inv = 1.0 / (10000.0 ** (np.arange(0, HD, 2, dtype=np.float32) / HD))
    fr = p[:, None] * inv[None, :]                       # (S, 32)
    emb = np.concatenate([fr, fr], axis=1)               # (S, 64)
    cosT = np.cos(emb).T.astype(np.float32)              # (64, S)
    sinT = np.sin(emb).T.astype(np.float32)
    # pre-shifted signed sin: multiplied at src rows, then shifted to dst
    ss_pre = np.concatenate([sinT[32:64], -sinT[0:32]], axis=0)  # (64, S)
    kcos = np.tile(cosT, (2, 1)).astype(bf)
    ksin = np.tile(ss_pre, (2, 1)).astype(bf)
    qcos = (np.tile(cosT, (2, 1)) * SCALE).astype(bf)
    qsin = (np.tile(ss_pre, (2, 1)) * SCALE).astype(bf)

    # additive causal mask of a diagonal 128x128 block as a rank-128 matmul:
    # (mska^T msknb)[k, q] = NEG iff k > q (exactly one product per entry).
    NEGv = float(mask.min())                             # -1e9
    r = np.arange(128)
    mska = (r[:, None] <= r[None, :]).astype(bf)         # A[r, k] = r <= k
    msknb = (NEGv * (r[:, None] == r[None, :] + 1)).astype(bf)

    in_maps = []
    for c in range(N_CORES):
        b, hg = c // 4, c % 4
        bv_slice = bv[128 * hg:128 * (hg + 1)]           # 2 kv heads x 64
        bvrep = np.tile(bv_slice[None, :], (128, 1)).astype(np.float32)
        in_maps.append({
            "xt": np.ascontiguousarray(x[b].T).astype(bf),
            "wqt": np.ascontiguousarray(Wq[512 * hg:512 * (hg + 1), :].T).astype(bf),
            "wkt": np.ascontiguousarray(Wk[128 * hg:128 * (hg + 1), :].T).astype(bf),
            "wvt": np.ascontiguousarray(Wv[128 * hg:128 * (hg + 1), :].T).astype(bf),
            "wot": np.ascontiguousarray(Wo[512 * hg:512 * (hg + 1), :].T).astype(bf),
            "bq": np.ascontiguousarray(bq[512 * hg:512 * (hg + 1)])[:, None],
            "bk": np.ascontiguousarray(bk[128 * hg:128 * (hg + 1)])[:, None],
            "bvrep": bvrep,
            "bo": np.ascontiguousarray(bo[512 * hg:512 * (hg + 1)])[:, None],
            "qcos": qcos, "qsin": qsin, "kcos": kcos, "ksin": ksin,
            "mska": mska, "msknb": msknb,
        })

    if _COMPILED is None:
        _install_profile_shim()
        _COMPILED = _build()

    res = bass_utils.run_bass_kernel_spmd(
        _COMPILED, in_maps, core_ids=list(range(N_CORES)), trace=TRACE)
    LAST_EXEC_NS = res.exec_time_ns

    outb = []
    for b in range(B):
        yt = np.concatenate([res.results[4 * b + hg]["out"]
                             for hg in range(4)], axis=0)   # [2048 oc, 2048 q]
        outb.append(yt.T)
    return np.stack(outb).astype(np.float32)


# revision 40
# speedup vs baseline: 1.0393x; 1.0393x over previous
"""Distributed GQA attention (B=2, S=2048, H=2048, 32 heads / 8 KV heads,
RoPE, causal) on 8 TRN2 NeuronCores.

Sharding: core c -> (batch b = c//4, head-group hg = c%4).
Each core computes q-heads [8hg, 8hg+8) and kv-heads [2hg, 2hg+2) of its
batch, runs attention locally (GQA groups stay on-core), then the four
cores of a batch AllGather their attention outputs (bf16) and each
computes a disjoint 512-column slice of the output projection.

Schedule (v2): attention runs kt-outer over 512-wide query chunks so
score/AV matmuls stream 512 columns (full PE rate) with exact causal
widths; softmax masking is a binary multiply on the Pool engine after
exp; the per-head normalization uses reciprocal_approx_fast and runs on
Pool.  Q-projection chunks and O-projection work are interleaved into
attention's scalar-bound bubbles, and the AllGather is split into four
512-column chunks so only the last O-proj chunk is a serial tail.
"""
import os
import sys

sys.path.insert(0, "/opt/trn_rl_repo")

import numpy as np
import ml_dtypes

import concourse.bass as bass
import concourse.mybir as mybir
import concourse.tile as tile
from concourse import bacc
from concourse import bass_utils

BF16 = mybir.dt.bfloat16
F32 = mybir.dt.float32
ADD = mybir.AluOpType.add
MULT = mybir.AluOpType.mult

B, S, H = 2, 2048, 2048
NH, NKV, HD = 32, 8, 64
SCALE = HD ** -0.5
RG = [[0, 1, 2, 3], [4, 5, 6, 7]]
N_CORES = 8
NT = S // 128          # 16 seq tiles
HT = H // 128          # 16 hidden tiles

TRACE = os.environ.get("BASS_KERNEL_TRACE", "0") == "1"
LAST_EXEC_NS = None
_COMPILED = None


def _install_profile_shim():
    import types
    try:
        from trn_agent_boot.trn_boot import _ntff_profile_via_ctypes
    except ImportError:
        return
    hook = _ntff_profile_via_ctypes("/opt/axon/libaxon_pjrt.so")
    mod = types.ModuleType("antenv.axon_hooks")
    mod.get_axon_ntff_profile_hook = lambda: hook
    mod.set_axon_ntff_profile_hook = lambda h: None
    sys.modules["antenv.axon_hooks"] = mod
    bass_utils.upload_artifacts = lambda tmpdir: tmpdir


def _build():
    nc = bacc.Bacc("TRN2", target_bir_lowering=False, debug=False,
                   num_devices=N_CORES)

    xt = nc.dram_tensor("xt", [H, S], BF16, kind="ExternalInput")
    wqt = nc.dram_tensor("wqt", [H, 512], BF16, kind="ExternalInput")
    wkt = nc.dram_tensor("wkt", [H, 128], BF16, kind="ExternalInput")
    wvt = nc.dram_tensor("wvt", [H, 128], BF16, kind="ExternalInput")
    wot = nc.dram_tensor("wot", [H, 512], BF16, kind="ExternalInput")
    bq = nc.dram_tensor("bq", [512, 1], F32, kind="ExternalInput")
    bk = nc.dram_tensor("bk", [128, 1], F32, kind="ExternalInput")
    bvrep = nc.dram_tensor("bvrep", [128, 128], F32, kind="ExternalInput")
    bo = nc.dram_tensor("bo", [512, 1], F32, kind="ExternalInput")
    qcos = nc.dram_tensor("qcos", [128, S], BF16, kind="ExternalInput")
    qsin = nc.dram_tensor("qsin", [128, S], BF16, kind="ExternalInput")
    kcos = nc.dram_tensor("kcos", [128, S], BF16, kind="ExternalInput")
    ksin = nc.dram_tensor("ksin", [128, S], BF16, kind="ExternalInput")
    mska = nc.dram_tensor("mska", [128, 128], BF16, kind="ExternalInput")
    msknb = nc.dram_tensor("msknb", [128, 128], BF16, kind="ExternalInput")
    out = nc.dram_tensor("out", [512, S], F32, kind="ExternalOutput")

    Exp = mybir.ActivationFunctionType.Exp

    from contextlib import ExitStack
    with tile.TileContext(nc) as tc:
        with ExitStack() as stk:
            ep = stk.enter_context
            big = ep(tc.tile_pool(name="big", bufs=16))     # xt / gathered o
            wpool = ep(tc.tile_pool(name="w", bufs=16))     # wqt / wot
            wkpool = ep(tc.tile_pool(name="wk", bufs=16))
            wvpool = ep(tc.tile_pool(name="wv", bufs=16))
            qpool = ep(tc.tile_pool(name="qt", bufs=4))
            kpool = ep(tc.tile_pool(name="kt", bufs=5))
            vpool = ep(tc.tile_pool(name="vv", bufs=16))
            opool = ep(tc.tile_pool(name="ot", bufs=4))
            tabpool = ep(tc.tile_pool(name="tab", bufs=4))
            mkpool = ep(tc.tile_pool(name="mk", bufs=1))
            ropepool = ep(tc.tile_pool(name="rope", bufs=6))
            expool = ep(tc.tile_pool(name="exp", bufs=4))
            nrmpool = ep(tc.tile_pool(name="nrm", bufs=2))
            ypool = ep(tc.tile_pool(name="yy", bufs=2))
            bpool = ep(tc.tile_pool(name="bias", bufs=12))
            pp = ep(tc.tile_pool(name="pp", bufs=2, space="PSUM"))
            scp = ep(tc.tile_pool(name="sc", bufs=2, space="PSUM"))
            avp = ep(tc.tile_pool(name="av", bufs=4, space="PSUM"))
            dram = ep(tc.tile_pool(name="dram", bufs=1, space="DRAM"))

            # ---------- input loads (weights for K/V first, then xt) ----------
            wk_sb, wv_sb = [], []
            for t in range(HT):
                k_t = wkpool.tile([128, 128], BF16, name=f"wk{t}", tag="wk")
                nc.sync.dma_start(out=k_t[:, :], in_=wkt[128 * t:128 * (t + 1), :])
                wk_sb.append(k_t)
                v_t = wvpool.tile([128, 128], BF16, name=f"wv{t}", tag="wv")
                nc.sync.dma_start(out=v_t[:, :], in_=wvt[128 * t:128 * (t + 1), :])
                wv_sb.append(v_t)
            xt_sb = []
            for t in range(HT):
                x_t = big.tile([128, S], BF16, name=f"xt{t}", tag="big")
                nc.sync.dma_start(out=x_t[:, :], in_=xt[128 * t:128 * (t + 1), :])
                xt_sb.append(x_t)
            wq_sb = []
            for t in range(HT):
                q_t = wpool.tile([128, 512], BF16, name=f"wq{t}", tag="w")
                nc.sync.dma_start(out=q_t[:, :], in_=wqt[128 * t:128 * (t + 1), :])
                wq_sb.append(q_t)
            kcos_sb = tabpool.tile([128, S], BF16, name="kcos", tag="tab")
            nc.sync.dma_start(out=kcos_sb[:, :], in_=kcos[:, :])
            ksin_sb = tabpool.tile([128, S], BF16, name="ksin", tag="tab")
            nc.sync.dma_start(out=ksin_sb[:, :], in_=ksin[:, :])
            qcos_sb = tabpool.tile([128, S], BF16, name="qcos", tag="tab")
            nc.sync.dma_start(out=qcos_sb[:, :], in_=qcos[:, :])
            qsin_sb = tabpool.tile([128, S], BF16, name="qsin", tag="tab")
            nc.sync.dma_start(out=qsin_sb[:, :], in_=qsin[:, :])
            mska_sb = mkpool.tile([128, 128], BF16, name="mska", tag="mka")
            nc.sync.dma_start(out=mska_sb[:, :], in_=mska[:, :])
            msknb_sb = mkpool.tile([128, 128], BF16, name="msknb", tag="mkb")
            nc.sync.dma_start(out=msknb_sb[:, :], in_=msknb[:, :])
            bq_sb, bo_sb = [], []
            for o in range(4):
                b_t = bpool.tile([128, 1], F32, name=f"bq{o}", tag="bias")
                nc.sync.dma_start(out=b_t[:, :], in_=bq[128 * o:128 * (o + 1), :])
                bq_sb.append(b_t)
            bk_sb = bpool.tile([128, 1], F32, name="bk", tag="bias")
            nc.sync.dma_start(out=bk_sb[:, :], in_=bk[:, :])
            bvrep_sb = bpool.tile([128, 128], F32, name="bvrep", tag="bvrep")
            nc.sync.dma_start(out=bvrep_sb[:, :], in_=bvrep[:, :])
            for o in range(4):
                b_t = bpool.tile([128, 1], F32, name=f"bo{o}", tag="bias")
                nc.sync.dma_start(out=b_t[:, :], in_=bo[128 * o:128 * (o + 1), :])
                bo_sb.append(b_t)

            def rope(psum, bias_ap, cos_sb, sin_sb, c, out_ap,
                     dve_shift=False):
                """out = (psum+bias)*cos + shift32((psum+bias)*sin_pre).

                dve_shift: do the rotate-half shift as 4 partition-shifted
                DVE adds instead of SBUF-SBUF DMAs — used for ropes emitted
                during attention, where the shift DMAs would queue behind
                AllGather traffic and stall the in-order PE queue."""
                cs = slice(512 * c, 512 * (c + 1))
                tcos = ropepool.tile([128, 512], F32, name="tcos", tag="rope")
                nc.vector.scalar_tensor_tensor(
                    tcos[:, :], psum[:, :], bias_ap, cos_sb[:, cs],
                    op0=ADD, op1=MULT)
                if dve_shift:
                    tsh = ropepool.tile([128, 512], F32, name="tsh",
                                        tag="rope")
                    for d, s in ((0, 32), (32, 0), (64, 96), (96, 64)):
                        nc.vector.scalar_tensor_tensor(
                            tsh[d:d + 32, :], psum[s:s + 32, :],
                            bias_ap[s:s + 32, :], sin_sb[s:s + 32, cs],
                            op0=ADD, op1=MULT)
                    nc.vector.tensor_tensor(out_ap, tcos[:, :], tsh[:, :],
                                            ADD)
                    return
                tsin = ropepool.tile([128, 512], F32, name="tsin", tag="rope")
                nc.vector.scalar_tensor_tensor(
                    tsin[:, :], psum[:, :], bias_ap, sin_sb[:, cs],
                    op0=ADD, op1=MULT)
                tsh = ropepool.tile([128, 512], F32, name="tsh", tag="rope")
                for d, s in ((0, 32), (32, 0), (64, 96), (96, 64)):
                    nc.sync.dma_start(out=tsh[d:d + 32, :], in_=tsin[s:s + 32, :])
                nc.vector.tensor_tensor(out_ap, tcos[:, :], tsh[:, :], ADD)

            # ---------- K projection (t-outer: streams behind the xt DMA) ----
            kp = []
            for c in range(4):
                pool = avp if c < 2 else pp
                kp.append(pool.tile([128, 512], F32, name=f"psk{c}",
                                    tag="av" if c < 2 else "pp"))
            for t in range(HT):
                for c in range(4):
                    nc.tensor.matmul(kp[c][:, :], wk_sb[t][:, :],
                                     xt_sb[t][:, 512 * c:512 * (c + 1)],
                                     start=(t == 0), stop=(t == HT - 1))
            kT_sb = kpool.tile([128, S], BF16, name="kT", tag="kt")
            # zero-padded K per (kv head, row band): [K_g; 0] for even heads,
            # [0; K_g] for odd heads.  Score matmuls then contract the full
            # 128 partitions of qT (the other head's rows hit the zero band),
            # streaming at full PE rate instead of the 64-deep half rate.
            # Memsets are emitted first: the DVE is idle during the input
            # DMA, and they must not queue behind the rope chain.
            kTe, kTo = [], []
            for g in range(2):
                e_t = kpool.tile([128, S], BF16, name=f"kTe{g}", tag="kt")
                nc.vector.memset(e_t[64:128, :], 0.0)
                kTe.append(e_t)
                o_t = kpool.tile([128, S], BF16, name=f"kTo{g}", tag="kt")
                nc.vector.memset(o_t[0:64, :], 0.0)
                kTo.append(o_t)
            for c in range(4):
                rope(kp[c], bk_sb[:, :], kcos_sb, ksin_sb, c,
                     kT_sb[:, 512 * c:512 * (c + 1)])
            for g in range(2):
                nc.sync.dma_start(out=kTe[g][0:64, :],
                                  in_=kT_sb[64 * g:64 * g + 64, :])
                nc.sync.dma_start(out=kTo[g][64:128, :],
                                  in_=kT_sb[64 * g:64 * g + 64, :])

            # ---------- Q projection (per 512-col chunk, generator) ----------
            qT_sb = []
            for o in range(4):
                q_t = qpool.tile([128, S], BF16, name=f"qT{o}", tag="qt")
                qT_sb.append(q_t)

            def qproj_gen(c):
                for o in range(4):
                    # chunk 0 runs at startup while pp slots still wait on
                    # the K-proj ropes for c2/c3 — alternate pools there.
                    pool, tg = ((avp, "av") if c == 0 and o < 2 else
                                (pp, "pp"))
                    psq = pool.tile([128, 512], F32, name=f"psq{c}_{o}",
                                    tag=tg)
                    for t in range(HT):
                        nc.tensor.matmul(psq[:, :],
                                         wq_sb[t][:, 128 * o:128 * (o + 1)],
                                         xt_sb[t][:, 512 * c:512 * (c + 1)],
                                         start=(t == 0), stop=(t == HT - 1))
                        yield
                    rope(psq, bq_sb[o][:, :], qcos_sb, qsin_sb, c,
                         qT_sb[o][:, 512 * c:512 * (c + 1)])
                    yield

            def drain(gen):
                for _ in gen:
                    pass

            # ---------- V projection (layout [rows, oc], 65-strided + ones) --
            v_sb = []

            def vproj_gen():
                for rt in range(NT):
                    v_t = vpool.tile([128, 130], BF16, name=f"v{rt}", tag="v")
                    nc.vector.memset(
                        v_t[:, :].rearrange("p (m c) -> p m c", c=65)[:, :, 64:65], 1.0)
                    ps = scp.tile([128, 128], F32, name="psv", tag="sc")
                    for t in range(HT):
                        nc.tensor.matmul(ps[:, :],
                                         xt_sb[t][:, 128 * rt:128 * (rt + 1)],
                                         wv_sb[t][:, :],
                                         start=(t == 0), stop=(t == HT - 1))
                    # v + bv: with normalized softmax weights, adding bv to V
                    # is exactly the attention-output V-bias.
                    nc.vector.tensor_tensor(
                        v_t[:, :].rearrange("p (m c) -> p m c", c=65)[:, :, 0:64],
                        ps[:, :].rearrange("p (m c) -> p m c", c=64),
                        bvrep_sb[:, :].rearrange("p (m c) -> p m c", c=64), ADD)
                    v_sb.append(v_t)
                    yield

            # round-robin Q chunk 0 with V so the DVE rope-Q and V-bias work
            # interleave and early v tiles are ready when attention starts
            gens0 = [qproj_gen(0), vproj_gen()]
            while gens0:
                for g in list(gens0):
                    try:
                        next(g)
                    except StopIteration:
                        gens0.remove(g)

            # ---------- attention + output projection ----------
            oT_sb = []
            for j in range(4):
                o_t = opool.tile([128, S], BF16, name=f"oT{j}", tag="ot")
                oT_sb.append(o_t)

            bounce = [dram.tile([512, 512], BF16, name=f"bounce{g}")
                      for g in range(4)]
            gath = [dram.tile([4, 512, 512], BF16, name=f"gath{g}")
                    for g in range(4)]
            wo_sb = [None] * HT

            def wo_gen():
                for t in range(HT):
                    w_t = wpool.tile([128, 512], BF16, name=f"wo{t}", tag="w")
                    nc.sync.dma_start(
                        out=w_t[:, :], in_=wot[128 * t:128 * (t + 1), :])
                    wo_sb[t] = w_t
                    yield

            def emit_gather(g):
                for jj in range(4):
                    nc.sync.dma_start(
                        out=bounce[g][128 * jj:128 * (jj + 1), :],
                        in_=oT_sb[jj][:, 512 * g:512 * (g + 1)])
                nc.gpsimd.collective_compute(
                    "AllGather", mybir.AluOpType.bypass, replica_groups=RG,
                    ins=[bounce[g][:, :].opt()],
                    outs=[gath[g][:, :, :].opt()])

            def oproj_gen(g):
                gview = gath[g][:, :, :].rearrange("g i q -> (g i) q")
                gsb = []
                for t in range(HT):
                    g_t = big.tile([128, 512], BF16, name=f"g{g}_{t}", tag="big")
                    nc.sync.dma_start(out=g_t[:, :],
                                      in_=gview[128 * t:128 * (t + 1), :])
                    gsb.append(g_t)
                yield
                for o in range(4):
                    ps = pp.tile([128, 512], F32, name=f"psy{g}_{o}", tag="pp")
                    for t in range(HT):
                        nc.tensor.matmul(
                            ps[:, :], wo_sb[t][:, 128 * o:128 * (o + 1)],
                            gsb[t][:, :],
                            start=(t == 0), stop=(t == HT - 1))
                        yield
                    y_t = ypool.tile([128, 512], F32, name="y", tag="y")
                    nc.vector.tensor_scalar_add(y_t[:, :], ps[:, :],
                                                bo_sb[o][:, :])
                    nc.sync.dma_start(
                        out=out[128 * o:128 * (o + 1),
                                512 * g:512 * (g + 1)],
                        in_=y_t[:, :])
                    yield

            def epilogue(j, c, av_e, av_o):
                cs = slice(512 * c, 512 * (c + 1))
                # partition shifts must be 32-aligned: land den_e at row 32
                # and den_o at row 64 of one tile (other rows are av values,
                # initialized but unused), then one reciprocal covers both.
                den = nrmpool.tile([65, 512], F32, name="den", tag="den")
                nc.vector.tensor_copy(den[0:64, :], av_e[0:64, :])    # filler
                nc.vector.tensor_copy(den[0:1, :], av_e[64:65, :])    # e @ 0
                nc.vector.tensor_copy(den[64:65, :], av_o[64:65, :])  # o @ 64
                rec = nrmpool.tile([65, 512], F32, name="rec", tag="rec")
                nc.vector.reciprocal(rec[:, :], den[:, :])
                # partition_broadcast reads partition 0 of its source on hw:
                # stage the o-head reciprocal down to partition 0.
                rec2 = nrmpool.tile([1, 512], F32, name="rec2", tag="rec2")
                nc.vector.tensor_copy(rec2[0:1, :], rec[64:65, :])
                for par, avt in ((0, av_e), (1, av_o)):
                    bcast = nrmpool.tile([64, 512], F32, name="bcast",
                                         tag="bcast")
                    nc.gpsimd.partition_broadcast(bcast[:, :],
                                                  rec[0:1, :] if par == 0
                                                  else rec2[0:1, :],
                                                  channels=64)
                    nc.vector.tensor_tensor(
                        oT_sb[j][64 * par:64 * par + 64, cs],
                        avt[0:64, :], bcast[:, :], MULT)

            def attn_chunk(c, gens):
                """Attention for q columns [512c, 512c+512), kt-outer.
                gens: generators of extra work interleaved into the PE
                bubbles (attention here is scalar-engine bound)."""
                n_iters = 4 * (4 * c + 4)
                n_done = 0

                def interleave(frac_done):
                    nonlocal n_done
                    # keep emitted fraction of gens ~ fraction of iters done
                    while gens and n_done < frac_done * _gen_total[0]:
                        try:
                            next(gens[0])
                            n_done += 1
                        except StopIteration:
                            gens.pop(0)

                _gen_total = [0]
                # count yields cheaply: assume caller passes (gen, size) pairs
                sized = gens
                gens = [g for g, n in sized]
                _gen_total[0] = sum(n for g, n in sized)

                it = 0
                last = 4 * c + 3
                for j in range(4):
                    kv = j // 2
                    ke = kTe[kv]
                    ko = kTo[kv]
                    av_e = avp.tile([65, 512], F32, name=f"av{c}{j}e", tag="av")
                    av_o = avp.tile([65, 512], F32, name=f"av{c}{j}o", tag="av")
                    for kt in range(last + 1):
                        off = 128 * (kt - 4 * c) if kt >= 4 * c else 0
                        w = 512 - off
                        qsl = slice(512 * c + off, 512 * (c + 1))
                        ks = slice(128 * kt, 128 * (kt + 1))
                        diag = kt >= 4 * c
                        se = scp.tile([128, 512], F32, name="se", tag="sc")
                        so = scp.tile([128, 512], F32, name="so", tag="sc")
                        nc.tensor.matmul(se[:, 0:w], ke[:, ks],
                                         qT_sb[j][:, qsl],
                                         start=True, stop=not diag)
                        if diag:
                            # additive causal mask of the diagonal 128x128
                            # block as a matmul: (A^T B)[k,q] = NEG iff k > q
                            nc.tensor.matmul(se[:, 0:128], mska_sb[:, :],
                                             msknb_sb[:, :],
                                             start=False, stop=True)
                        nc.tensor.matmul(so[:, 0:w], ko[:, ks],
                                         qT_sb[j][:, qsl],
                                         start=True, stop=not diag)
                        if diag:
                            nc.tensor.matmul(so[:, 0:128], mska_sb[:, :],
                                             msknb_sb[:, :],
                                             start=False, stop=True)
                        ebuf = expool.tile([128, 512], BF16, name="ebuf",
                                           tag="exp")
                        obuf = expool.tile([128, 512], BF16, name="obuf",
                                           tag="exp")
                        nc.scalar.activation(ebuf[:, 0:w], se[:, 0:w], Exp)
                        nc.scalar.activation(obuf[:, 0:w], so[:, 0:w], Exp)
                        vs = v_sb[kt][:, 65 * kv:65 * kv + 65]
                        nc.tensor.matmul(av_e[:, off:512], vs, ebuf[:, 0:w],
                                         start=(kt == 0), stop=(kt == last))
                        nc.tensor.matmul(av_o[:, off:512], vs, obuf[:, 0:w],
                                         start=(kt == 0), stop=(kt == last))
                        it += 1
                        interleave(it / n_iters)
                    epilogue(j, c, av_e, av_o)
                # drain leftovers
                for g in gens:
                    drain(g)

            QP_N = 4 * (HT + 1)          # yields in qproj_gen
            OP_N = 1 + 4 * (HT + 1)      # yields in oproj_gen
            attn_chunk(0, [(qproj_gen(1), QP_N)])
            emit_gather(0)
            attn_chunk(1, [(qproj_gen(2), QP_N)])
            emit_gather(1)
            attn_chunk(2, [(qproj_gen(3), QP_N), (wo_gen(), HT)])
            emit_gather(2)
            attn_chunk(3, [(oproj_gen(0), OP_N)])
            drain(oproj_gen(1))
            drain(oproj_gen(2))
            emit_gather(3)
            drain(oproj_gen(3))

    nc.compile()
    return nc


def kernel(**inputs):
    global _COMPILED, LAST_EXEC_NS
    x = np.asarray(inputs["hidden_states"], dtype=np.float32)
    mask = np.asarray(inputs["attention_mask"], dtype=np.float32)
    pos = np.asarray(inputs["position_ids"])
    Wq = np.asarray(inputs["Wq"], dtype=np.float32)
    bq = np.asarray(inputs["bq"], dtype=np.float32)
    Wk = np.asarray(inputs["Wk"], dtype=np.float32)
    bk = np.asarray(inputs["bk"], dtype=np.float32)
    Wv = np.asarray(inputs["Wv"], dtype=np.float32)
    bv = np.asarray(inputs["bv"], dtype=np.float32)
    Wo = np.asarray(inputs["Wo"], dtype=np.float32)
    bo = np.asarray(inputs["bo"], dtype=np.float32)

    bf = ml_dtypes.bfloat16
    # rope tables (from the position_ids input)
    p = pos[0].astype(np.float32)
    inv = 1.0 / (10000.0 ** (np.arange(0, HD, 2, dtype=np.float32) / HD))
    fr = p[:, None] * inv[None, :]                       # (S, 32)
    emb = np.concatenate([fr, fr], axis=1)               # (S, 64)
    cosT = np.cos(emb).T.astype(np.float32)              # (64, S)
    sinT = np.sin(emb).T.astype(np.float32)
    # pre-shifted signed sin: multiplied at src rows, then shifted to dst
    ss_pre = np.concatenate([sinT[32:64], -sinT[0:32]], axis=0)  # (64, S)
    kcos = np.tile(cosT, (2, 1)).astype(bf)
    ksin = np.tile(ss_pre, (2, 1)).astype(bf)
    qcos = (np.tile(cosT, (2, 1)) * SCALE).astype(bf)
    qsin = (np.tile(ss_pre, (2, 1)) * SCALE).astype(bf)

    # additive causal mask of a diagonal 128x128 block as a rank-128 matmul:
    # (mska^T msknb)[k, q] = NEG iff k > q (exactly one product per entry).
    NEGv = float(mask.min())                             # -1e9
    r = np.arange(128)
    mska = (r[:, None] <= r[None, :]).astype(bf)         # A[r, k] = r <= k
    msknb = (NEGv * (r[:, None] == r[None, :] + 1)).astype(bf)

    in_maps = []
    for c in range(N_CORES):
        b, hg = c // 4, c % 4
        bv_slice = bv[128 * hg:128 * (hg + 1)]           # 2 kv heads x 64
        bvrep = np.tile(bv_slice[None, :], (128, 1)).astype(np.float32)
        in_maps.append({
            "xt": np.ascontiguousarray(x[b].T).astype(bf),
            "wqt": np.ascontiguousarray(Wq[512 * hg:512 * (hg + 1), :].T).astype(bf),
            "wkt": np.ascontiguousarray(Wk[128 * hg:128 * (hg + 1), :].T).astype(bf),
            "wvt": np.ascontiguousarray(Wv[128 * hg:128 * (hg + 1), :].T).astype(bf),
            "wot": np.ascontiguousarray(Wo[512 * hg:512 * (hg + 1), :].T).astype(bf),
            "bq": np.ascontiguousarray(bq[512 * hg:512 * (hg + 1)])[:, None],
            "bk": np.ascontiguousarray(bk[128 * hg:128 * (hg + 1)])[:, None],
            "bvrep": bvrep,
            "bo": np.ascontiguousarray(bo[512 * hg:512 * (hg + 1)])[:, None],
            "qcos": qcos, "qsin": qsin, "kcos": kcos, "ksin": ksin,
            "mska": mska, "msknb": msknb,
        })

    if _COMPILED is None:
        _install_profile_shim()
        _COMPILED = _build()

    res = bass_utils.run_bass_kernel_spmd(
        _COMPILED, in_maps, core_ids=list(range(N_CORES)), trace=TRACE)
    LAST_EXEC_NS = res.exec_time_ns

    outb = []
    for b in range(B):
        yt = np.concatenate([res.results[4 * b + hg]["out"]
                             for hg in range(4)], axis=0)   # [2048 oc, 2048 q]
        outb.append(yt.T)
    return np.stack(outb).astype(np.float32)
